# revision 1
# baseline (speedup 1.0000x reference)
"""Trainium2 Bass kernel for nn_Caption (LSTM caption decoder).

Distribution: pure data-parallel over batch (128 -> 8 cores x 16), no
collectives. Per core: x0 projection GEMM, embedding gather (device),
input-gate GEMM, 40-step LSTM recurrence, vocab GEMM [640,512]@[512,10000].

Layout strategy: all GEMM operands bf16 (fp32 PSUM accumulation); weights
host-transposed so the contraction dim lands on partitions; outputs
produced in T-layout (feature on partitions) so biases fuse into ACT
copies as per-partition bias. LSTM runs B-layout (batch on partitions)
with per-step h transposed via PE into hiddensT, which is consumed
directly by the vocab GEMM. xg is injected into the gates PSUM via
identity matmuls (t-blocks padded to 32 partitions for alignment).
"""
import sys

sys.path.insert(0, "/opt/trn_rl_repo")

import numpy as np
import ml_dtypes

import concourse.bass as bass
import concourse.tile as tile
from concourse import bacc, mybir
from concourse.bass_utils import run_bass_kernel_spmd
from concourse.masks import make_identity

BF = mybir.dt.bfloat16
F32 = mybir.dt.float32
I32 = mybir.dt.int32
bfnp = ml_dtypes.bfloat16

B, F, E, H, V, T = 128, 1536, 512, 512, 10000, 40
NCORES = 8
BC = B // NCORES          # 16 batch rows per core
TB = 32                   # padded t-block width (partition alignment)
NTB = T * TB              # 1280 padded (t,b) columns
NB = T * BC               # 640 real (t,b) columns
G4 = 4 * H                # 2048 gate dims, order [i, f, o, g]
VP = 10240               # padded vocab (80 tiles of 128, 20 quads)
NVT = VP // 128           # 80 vocab tiles
NVQ = NVT // 4            # 20 vocab quads

_CACHE = {}


def _build():
    if "nc" in _CACHE:
        return _CACHE["nc"]
    nc = bacc.Bacc("TRN2", target_bir_lowering=False, debug=False,
                   num_devices=NCORES)

    featT_d = nc.dram_tensor("featT", [F, BC], BF, kind="ExternalInput")
    idx_d = nc.dram_tensor("idx", [NTB, 1], I32, kind="ExternalInput")
    emb_d = nc.dram_tensor("embt", [V, E], BF, kind="ExternalInput")
    WinT_d = nc.dram_tensor("WinT", [128, 12, E], BF, kind="ExternalInput")
    WihT_d = nc.dram_tensor("WihT", [128, 4, G4], BF, kind="ExternalInput")
    WhhT_d = nc.dram_tensor("WhhT", [128, 4, G4], BF, kind="ExternalInput")
    bcomb_d = nc.dram_tensor("bcomb", [G4], F32, kind="ExternalInput")
    bin_d = nc.dram_tensor("bin", [E], F32, kind="ExternalInput")
    ident_d = nc.dram_tensor("ident", [128, 128], BF, kind="ExternalInput")
    WoutTt_d = nc.dram_tensor("WoutTt", [NVQ, 128, 4, 512], BF,
                              kind="ExternalInput")
    out_d = nc.dram_tensor("out_q", [3, NVQ * 4, 128, 256], F32,
                           kind="ExternalOutput")

    with tile.TileContext(nc) as tc:
        with (
            tc.tile_pool(name="consts", bufs=1) as consts,
            tc.tile_pool(name="big", bufs=1) as big,
            tc.tile_pool(name="state", bufs=3) as state,
            tc.tile_pool(name="work", bufs=3) as work,
            tc.tile_pool(name="wpool", bufs=5) as wpool,
            tc.tile_pool(name="lpool", bufs=3) as lpool,
        ):
            # ---- index load + constants ----
            idx_sb = consts.tile([128, 10, 1], I32)
            nc.sync.dma_start(
                idx_sb[:], idx_d.ap().rearrange("(j p) o -> p j o", p=128))
            identb = consts.tile([128, 128], BF)
            nc.sync.dma_start(identb[:], ident_d.ap())

            WihT_sb = big.tile([128, 4, G4], BF, tag="wih")
            nc.sync.dma_start(WihT_sb[:], WihT_d.ap())
            WinT_sb = big.tile([128, 12, E], BF, tag="win")
            nc.sync.dma_start(WinT_sb[:], WinT_d.ap())
            featT_sb = consts.tile([128, 12, BC], BF)
            nc.sync.dma_start(
                featT_sb[:], featT_d.ap().rearrange("(k p) b -> p k b", p=128))
            WhhT_sb = big.tile([128, 4, G4], BF, tag="whh")
            nc.sync.dma_start(WhhT_sb[:], WhhT_d.ap())
            bias_bc = big.tile([128, G4], F32, tag="biasbc")
            nc.sync.dma_start(
                bias_bc[:],
                bass.AP(tensor=bcomb_d, offset=0, ap=[[0, 128], [1, G4]]))
            bin_sb = consts.tile([128, 4], F32)
            nc.sync.dma_start(
                bin_sb[:], bin_d.ap().rearrange("(k p) -> p k", p=128))


            # ---- embedding gather -> seqT (transposed via PE) ----
            seqT = big.tile([128, 4, NTB], BF, tag="seqT")
            with tc.tile_pool(name="psA", bufs=3, space="PSUM") as psA:
                for j in range(10):
                    gt = work.tile([128, E], BF, tag="gather")
                    nc.gpsimd.indirect_dma_start(
                        out=gt[:], out_offset=None, in_=emb_d.ap(),
                        in_offset=bass.IndirectOffsetOnAxis(
                            ap=idx_sb[:, j, :], axis=0))
                    for e in range(4):
                        pst = psA.tile([128, 128], BF, space="PSUM", tag="tr")
                        nc.tensor.transpose(
                            pst[:], gt[:, e * 128:(e + 1) * 128], identb[:])
                        nc.scalar.copy(
                            seqT[:, e, j * 128:(j + 1) * 128], pst[:])

                # ---- x0T = W_inT.T @ featT + b_in -> seqT[:, :, 0:BC] ----
                for m in range(4):
                    ps = psA.tile([128, BC], F32, space="PSUM", tag="x0")
                    for k in range(12):
                        nc.tensor.matmul(
                            ps[:], lhsT=WinT_sb[:, k, m * 128:(m + 1) * 128],
                            rhs=featT_sb[:, k, :],
                            start=(k == 0), stop=(k == 11))
                    nc.scalar.activation(
                        seqT[:, m, 0:BC], ps[:],
                        mybir.ActivationFunctionType.Identity,
                        bias=bin_sb[:, m:m + 1])

            # ---- LSTM + interleaved xg / vocab GEMM ----
            VWIN = [(0, 16), (16, 32), (32, 40)]
            hid_w = [big.tile([128, 4, t1 - t0, BC], BF, tag=f"hid{w}",
                              name=f"hid{w}")
                     for w, (t0, t1) in enumerate(VWIN)]
            xg_tiles = [big.tile([128, G4], BF, tag=f"xg{mt}", name=f"xg{mt}")
                        for mt in range(10)]

            def hid_of(t):
                for w, (t0, t1) in enumerate(VWIN):
                    if t < t1:
                        return hid_w[w], t - t0
                raise AssertionError

            lstm_ps = tc.tile_pool(name="psGates", bufs=1, space="PSUM")
            htr_ps = tc.tile_pool(name="psHtr", bufs=2, space="PSUM")
            xg_ps = tc.tile_pool(name="psXg", bufs=2, space="PSUM")
            voc_ps = tc.tile_pool(name="psVoc", bufs=2, space="PSUM")
            gpsum = lstm_ps.__enter__()
            tpsum = htr_ps.__enter__()
            xgpsum = xg_ps.__enter__()
            vpsum = [None]

            def emit_xg_mtile(mt):
                for n in range(4):
                    ps = xgpsum.tile([128, 512], F32, space="PSUM", tag="xgps")
                    for k in range(4):
                        nc.tensor.matmul(
                            ps[:],
                            lhsT=seqT[:, k, mt * 128:(mt + 1) * 128],
                            rhs=WihT_sb[:, k, n * 512:(n + 1) * 512],
                            start=(k == 0), stop=(k == 3))
                    nc.vector.tensor_add(
                        xg_tiles[mt][:, n * 512:(n + 1) * 512], ps[:],
                        bias_bc[:, n * 512:(n + 1) * 512])

            vunits = []
            ncopy = [0]

            def emit_vocab_unit(vtq, w):
                t0, t1 = VWIN[w]
                nb = (t1 - t0) * BC
                wt = wpool.tile([128, 4, 512], BF, tag="wout")
                nc.sync.dma_start(wt[:], WoutTt_d.ap()[vtq])
                lsb = lpool.tile([128, 4, 256], F32, tag="lout")
                for pair in range(2):
                    vps = vpsum[0].tile([128, 512], F32, space="PSUM",
                                        tag="vps")
                    for half in range(2):
                        sub = pair * 2 + half
                        hsl = slice(half * 256, half * 256 + nb)
                        for k in range(4):
                            nc.tensor.matmul(
                                vps[:, hsl],
                                lhsT=wt[:, k, sub * 128:(sub + 1) * 128],
                                rhs=hid_w[w][:, k, :, :],
                                start=(k == 0 and half == 0),
                                stop=(k == 3 and half == 1))
                    dst = lsb[:, 2 * pair:2 * pair + 2, 0:256] \
                        .rearrange("p s c -> p (s c)")
                    if pair == 0:
                        nc.scalar.copy(dst, vps[:])
                    else:
                        nc.vector.tensor_copy(dst, vps[:])
                nc.gpsimd.dma_start(
                    out_d.ap()[w, vtq * 4:(vtq + 1) * 4, :, :]
                    .rearrange("s p c -> p s c"),
                    lsb[:])

            emit_xg_mtile(0)
            emit_xg_mtile(1)

            HH = H // 2
            c_prev = None
            for t in range(T):
                if t == 16:
                    xg_ps.__exit__(None, None, None)
                    vpsum[0] = voc_ps.__enter__()
                mt, po = (t * TB) // 128, (t * TB) % 128
                gchunk = {}
                for n in (3, 0, 1, 2):
                    gchunk[n] = gpsum.tile(
                        [BC, 512], F32, space="PSUM", tag=f"gates{n}",
                        name=f"gates{n}")
                sig_i = state.tile([BC, H], F32, tag="sigi")
                sig_f = state.tile([BC, H], F32, tag="sigf")
                sig_o = state.tile([BC, H], F32, tag="sigo")
                g_t = state.tile([BC, H], F32, tag="g")
                act_of = {3: (g_t, mybir.ActivationFunctionType.Tanh),
                          0: (sig_i, mybir.ActivationFunctionType.Sigmoid),
                          1: (sig_f, mybir.ActivationFunctionType.Sigmoid),
                          2: (sig_o, mybir.ActivationFunctionType.Sigmoid)}
                for n in (3, 0, 1, 2):
                    ns = slice(n * 512, (n + 1) * 512)
                    nc.tensor.matmul(
                        gchunk[n][:],
                        lhsT=identb[po:po + BC, po:po + BC],
                        rhs=xg_tiles[mt][po:po + BC, ns],
                        start=True, stop=(t == 0),
                        tile_position=(po, 0))
                    if t > 0:
                        hsrc, trel = hid_of(t - 1)
                        for k in range(4):
                            nc.tensor.matmul(
                                gchunk[n][:],
                                lhsT=hsrc[:, k, trel, :],
                                rhs=WhhT_sb[:, k, ns],
                                start=False, stop=(k == 3))
                    dst, fn = act_of[n]
                    if n == 2:
                        nc.scalar.activation(dst[:, 0:256], gchunk[n][:, 0:256], fn)
                        nc.scalar.activation(dst[:, 256:512], gchunk[n][:, 256:512], fn)
                    else:
                        nc.scalar.activation(dst[:], gchunk[n][:], fn)

                hdst, trel = hid_of(t)
                c_new = [None, None]
                for half in range(2):
                    hs = slice(half * HH, (half + 1) * HH)
                    ig = state.tile([BC, HH], F32, tag=f"ig{half}")
                    nc.vector.tensor_mul(ig[:], sig_i[:, hs], g_t[:, hs])
                    cn = state.tile([BC, HH], F32, tag=f"c{half}")
                    if t == 0:
                        nc.vector.tensor_copy(cn[:], ig[:])
                    else:
                        cf = state.tile([BC, HH], F32, tag=f"cf{half}")
                        nc.vector.tensor_mul(
                            cf[:], sig_f[:, hs], c_prev[half][:])
                        nc.vector.tensor_add(cn[:], cf[:], ig[:])
                    c_new[half] = cn
                    tc_t = state.tile([BC, HH], F32, tag=f"tanhc{half}")
                    nc.scalar.activation(
                        tc_t[:], cn[:], mybir.ActivationFunctionType.Tanh)
                    h_bf = state.tile([BC, HH], BF, tag=f"h{half}")
                    nc.vector.tensor_mul(h_bf[:], sig_o[:, hs], tc_t[:])
                    pst = tpsum.tile([128, 2 * BC], BF, space="PSUM",
                                     tag="htr")
                    for e in range(2):
                        nc.tensor.transpose(
                            pst[:, e * BC:(e + 1) * BC],
                            h_bf[:, e * 128:(e + 1) * 128],
                            identb[0:BC, 0:BC])
                    dstap = hdst[:, 2 * half:2 * half + 2, trel, :]
                    srcap = pst[:].rearrange("p (k b) -> p k b", k=2)
                    if half == 0:
                        nc.scalar.copy(dstap, srcap)
                    else:
                        nc.vector.tensor_copy(dstap, srcap)
                c_prev = c_new

                # interleaved filler work
                if t < 16 and t % 2 == 0 and t // 2 + 2 < 10:
                    emit_xg_mtile(t // 2 + 2)
                for w, (t0, t1) in enumerate(VWIN):
                    if t == t1 - 1:
                        vunits.extend((vtq, w) for vtq in range(NVQ))
                if t >= 16:
                    if vunits:
                        emit_vocab_unit(*vunits.pop(0))

            voc_ps.__exit__(None, None, None)
            htr_ps.__exit__(None, None, None)
            lstm_ps.__exit__(None, None, None)

            # vocab tail with wide PSUM pool
            with tc.tile_pool(name="psVoc2", bufs=6, space="PSUM") as vp2:
                vpsum[0] = vp2
                while vunits:
                    emit_vocab_unit(*vunits.pop(0))

    nc.compile()
    _CACHE["nc"] = nc
    return nc


def kernel(features, seqs, lengths, W_in, b_in, emb, W_ih, W_hh, b_ih, b_hh,
           W_out, b_out):
    f32 = lambda x: np.asarray(x, dtype=np.float32)
    bf = lambda x: np.ascontiguousarray(f32(x)).astype(bfnp)
    features, seqs = f32(features), np.asarray(seqs).astype(np.int64)
    # gate order [i, f, o, g]
    perm = np.concatenate([np.arange(0, 2 * H), np.arange(3 * H, 4 * H),
                           np.arange(2 * H, 3 * H)])
    WinT = np.ascontiguousarray(
        bf(f32(W_in).T).reshape(12, 128, E).transpose(1, 0, 2))
    WihT = np.ascontiguousarray(
        bf(f32(W_ih).T)[:, perm].reshape(4, 128, G4).transpose(1, 0, 2))
    WhhT = np.ascontiguousarray(
        bf(f32(W_hh).T)[:, perm].reshape(4, 128, G4).transpose(1, 0, 2))
    bcomb = np.ascontiguousarray((f32(b_ih) + f32(b_hh))[perm])
    emb_b = bf(emb)
    WoutT = np.zeros((H, VP), dtype=bfnp)
    WoutT[:, :V] = bf(f32(W_out).T)
    # quad-tiled layout [vtq, p, k, v4]: element = WoutT[k*128+p, vtq*512+v4]
    WoutTt = np.ascontiguousarray(
        WoutT.reshape(4, 128, NVQ, 512).transpose(2, 1, 0, 3))
    ident_np = np.eye(128, dtype=bfnp)
    binp = f32(b_in)

    nc = _build()
    in_maps = []
    for c in range(NCORES):
        bs = slice(c * BC, (c + 1) * BC)
        featT = bf(features[bs].T)             # [F, BC]
        idx = np.zeros((T, TB), np.int64)
        idx[1:, :BC] = seqs[bs].T              # t-major, t=0 block dummy
        in_maps.append({
            "featT": featT,
            "idx": idx.reshape(NTB, 1).astype(np.int32),
            "embt": emb_b,
            "WinT": WinT, "WihT": WihT, "WhhT": WhhT,
            "bcomb": bcomb, "bin": binp, "ident": ident_np,
            "WoutTt": WoutTt,
        })
    _CACHE["last_in_maps"] = in_maps
    res = run_bass_kernel_spmd(nc, in_maps, list(range(NCORES)))
    out = np.empty((B, T, V), np.float32)
    wlens = [256, 256, 128]
    for c in range(NCORES):
        oq = res.results[c]["out_q"]           # [3, 80, 128, 256]
        parts = [oq[w].reshape(VP, 256)[:V, :wlens[w]] for w in range(3)]
        lt = np.concatenate(parts, axis=1)     # [V, 640]
        out[c * BC:(c + 1) * BC] = (
            lt.reshape(V, T, BC).transpose(2, 1, 0))
    bo = f32(b_out)
    if np.any(bo):
        out += bo
    return out



# revision 3
# speedup vs baseline: 1.1853x; 1.1853x over previous
"""Trainium2 Bass kernel for nn_Caption (LSTM caption decoder).

Distribution: pure data-parallel over batch (128 -> 8 cores x 16), no
collectives. Per core: x0 projection GEMM, embedding gather (device),
input-gate GEMM, 40-step LSTM recurrence, vocab GEMM.

v2 layout strategy (transposed gates): everything runs in T-layout
(feature/gate dims on partitions, (t,b) columns on free axis).
 - seqT [E, 640] built from PE transposes of gathered embeddings + x0.
 - xg^T [4H, 640] computed in 3 t-chunks (t0:8 before the LSTM; t8:24
   and t24:40 emitted as PE filler inside early LSTM steps).
 - LSTM gates computed transposed: per step 16 gate M-tiles [128, 16]
   = identity-inject of xg^T column block + 4 k-tile matmuls of
   W_hh^T @ h^T. N=16 matmuls issue back-to-back at ~27ns, so a step
   costs ~2.2us of PE vs ~4.9us in the B-layout version. Elementwise
   ops run full-width [128, 64]; h^T is written directly in the layout
   the vocab GEMM consumes (no per-step PE transposes).
 - vocab GEMM: W_out resident in SBUF (loaded once, 10.5MB bf16), 80
   v-tiles x 3 t-windows, outputs written bf16 (halves write traffic),
   interleaved into LSTM steps t>=16, tail drains after the loop.
"""
import sys

sys.path.insert(0, "/opt/trn_rl_repo")

import numpy as np
import ml_dtypes

import concourse.bass as bass
import concourse.tile as tile
from concourse import bacc, mybir
from concourse.bass_utils import run_bass_kernel_spmd

BF = mybir.dt.bfloat16
F32 = mybir.dt.float32
I32 = mybir.dt.int32
bfnp = ml_dtypes.bfloat16

B, F, E, H, V, T = 128, 1536, 512, 512, 10000, 40
NCORES = 8
BC = B // NCORES          # 16 batch rows per core
NB = T * BC               # 640 (t,b) columns, t-major
G4 = 4 * H                # 2048 gate dims, natural order [i, f, g, o]
NGT = G4 // 128           # 16 gate M-tiles
VP = 10240                # padded vocab
NVT = VP // 128           # 80 vocab M-tiles
XCH = [(0, 8), (8, 24), (24, 40)]     # xg^T t-chunks
VWIN = [(0, 16), (16, 32), (32, 40)]  # hidden/vocab t-windows

_CACHE = {}


def _build():
    if "nc" in _CACHE:
        return _CACHE["nc"]
    nc = bacc.Bacc("TRN2", target_bir_lowering=False, debug=False,
                   num_devices=NCORES)

    featT_d = nc.dram_tensor("featT", [F, BC], BF, kind="ExternalInput")
    idx_d = nc.dram_tensor("idx", [NB, 1], I32, kind="ExternalInput")
    emb_d = nc.dram_tensor("embt", [V, E], BF, kind="ExternalInput")
    WinT_d = nc.dram_tensor("WinT", [128, 12, 4, 128], BF,
                            kind="ExternalInput")
    WihT_d = nc.dram_tensor("WihT", [128, 4, NGT, 128], BF,
                            kind="ExternalInput")
    WhhT_d = nc.dram_tensor("WhhT", [128, 4, NGT, 128], BF,
                            kind="ExternalInput")
    WoutT_d = nc.dram_tensor("WoutT", [128, 4, NVT, 128], BF,
                             kind="ExternalInput")
    bcomb_d = nc.dram_tensor("bcomb", [G4], F32, kind="ExternalInput")
    bin_d = nc.dram_tensor("bin", [E], F32, kind="ExternalInput")
    ident_d = nc.dram_tensor("ident", [128, 128], BF, kind="ExternalInput")
    out_d = [nc.dram_tensor(f"out{w}", [NVT, 128, (t1 - t0) * BC], BF,
                            kind="ExternalOutput")
             for w, (t0, t1) in enumerate(VWIN)]

    AFT = mybir.ActivationFunctionType

    with tile.TileContext(nc) as tc:
        with (
            tc.tile_pool(name="consts", bufs=1) as consts,
            tc.tile_pool(name="big", bufs=1) as big,
            tc.tile_pool(name="state", bufs=2) as state,
            tc.tile_pool(name="gat", bufs=3) as gat,
            tc.tile_pool(name="vstage", bufs=4) as vstage,
        ):
            # ---- constants / small inputs ----
            idx_sb = consts.tile([128, 5, 1], I32)
            nc.sync.dma_start(
                idx_sb[:], idx_d.ap().rearrange("(j p) o -> p j o", p=128))
            identb = consts.tile([128, 128], BF)
            nc.sync.dma_start(identb[:], ident_d.ap())
            featT_sb = consts.tile([128, 12, BC], BF)
            nc.sync.dma_start(
                featT_sb[:], featT_d.ap().rearrange("(k p) b -> p k b", p=128))
            bcombT = consts.tile([128, NGT], F32)
            nc.sync.dma_start(
                bcombT[:], bcomb_d.ap().rearrange("(g p) -> p g", p=128))
            binT = consts.tile([128, 4], F32)
            nc.sync.dma_start(
                binT[:], bin_d.ap().rearrange("(m p) -> p m", p=128))

            # ---- weights ----
            WinT_sb = big.tile([128, 12, 4, 128], BF, tag="win")
            nc.sync.dma_start(WinT_sb[:], WinT_d.ap())
            WihT_sb = big.tile([128, 4, NGT, 128], BF, tag="wih")
            nc.sync.dma_start(WihT_sb[:], WihT_d.ap())
            WhhT_sb = big.tile([128, 4, NGT, 128], BF, tag="whh")
            nc.sync.dma_start(WhhT_sb[:], WhhT_d.ap())
            Wout_sb = big.tile([128, 4, NVT, 128], BF, tag="wout")
            for k in range(4):
                nc.sync.dma_start(Wout_sb[:, k, :, :], WoutT_d.ap()[:, k, :, :])

            # ---- big working tiles ----
            seqT = big.tile([128, 4, NB], BF, tag="seqT")
            xgc = [big.tile([128, NGT, (t1 - t0) * BC], BF, tag=f"xg{i}",
                            name=f"xg{i}")
                   for i, (t0, t1) in enumerate(XCH)]
            hid_w = [big.tile([128, 4, t1 - t0, BC], BF, tag=f"hid{w}",
                              name=f"hid{w}")
                     for w, (t0, t1) in enumerate(VWIN)]

            def hid_of(t):
                for w, (t0, t1) in enumerate(VWIN):
                    if t < t1:
                        return hid_w[w], t - t0
                raise AssertionError

            ps_early = tc.tile_pool(name="psEarly", bufs=2, space="PSUM")
            ps_gates = tc.tile_pool(name="psGates", bufs=2, space="PSUM")
            ps_voc = tc.tile_pool(name="psVoc", bufs=3, space="PSUM")
            pG = ps_gates.__enter__()
            pE = ps_early.__enter__()
            vpsum = [None]

            # ---- embedding gather -> seqT (transposed via PE) ----
            for j in range(5):
                gt_t = gat.tile([128, E], BF, tag="gather")
                nc.gpsimd.indirect_dma_start(
                    out=gt_t[:], out_offset=None, in_=emb_d.ap(),
                    in_offset=bass.IndirectOffsetOnAxis(
                        ap=idx_sb[:, j, :], axis=0))
                for e in range(4):
                    pst = pE.tile([128, 128], BF, space="PSUM", tag="tr")
                    nc.tensor.transpose(
                        pst[:], gt_t[:, e * 128:(e + 1) * 128], identb[:])
                    if j == 0:
                        # cols 0:16 belong to x0 (dummy gather rows)
                        nc.scalar.copy(seqT[:, e, BC:128], pst[:, BC:128])
                    else:
                        nc.scalar.copy(
                            seqT[:, e, j * 128:(j + 1) * 128], pst[:])

            # ---- x0^T = W_in @ features^T + b_in -> seqT[:, :, 0:BC] ----
            for m in range(4):
                ps = pE.tile([128, BC], F32, space="PSUM", tag="x0")
                for k in range(12):
                    nc.tensor.matmul(
                        ps[:], lhsT=WinT_sb[:, k, m, :],
                        rhs=featT_sb[:, k, :],
                        start=(k == 0), stop=(k == 11))
                nc.scalar.activation(
                    seqT[:, m, 0:BC], ps[:], AFT.Identity,
                    bias=binT[:, m:m + 1])

            # ---- xg^T chunk emitter ----
            def emit_xg(ci, gt):
                t0, t1 = XCH[ci]
                c0, n = t0 * BC, (t1 - t0) * BC
                ps = pE.tile([128, 512], F32, space="PSUM", tag="xg")
                for k in range(4):
                    nc.tensor.matmul(
                        ps[:, 0:n], lhsT=WihT_sb[:, k, gt, :],
                        rhs=seqT[:, k, c0:c0 + n],
                        start=(k == 0), stop=(k == 3))
                nc.scalar.activation(
                    xgc[ci][:, gt, :], ps[:, 0:n], AFT.Identity,
                    bias=bcombT[:, gt:gt + 1])

            for gt in range(NGT):
                emit_xg(0, gt)

            # ---- vocab unit emitter ----
            ncopy = [0]

            def emit_vunit(vt, w):
                t0, t1 = VWIN[w]
                n = (t1 - t0) * BC
                ps = vpsum[0].tile([128, 512], F32, space="PSUM", tag="v")
                for k in range(4):
                    nc.tensor.matmul(
                        ps[:, 0:n], lhsT=Wout_sb[:, k, vt, :],
                        rhs=hid_w[w][:, k, :, :],
                        start=(k == 0), stop=(k == 3))
                st = vstage.tile([128, 512], BF, tag="vs")
                if ncopy[0] % 2 == 0:
                    nc.scalar.copy(st[:, 0:n], ps[:, 0:n])
                else:
                    nc.vector.tensor_copy(st[:, 0:n], ps[:, 0:n])
                ncopy[0] += 1
                nc.gpsimd.dma_start(out_d[w].ap()[vt], st[:, 0:n])

            # ---- LSTM loop ----
            GO = (2, 0, 1, 3)  # gate order g, i, f, o (0=i,1=f,2=g,3=o)
            act_fn = {0: AFT.Sigmoid, 1: AFT.Sigmoid,
                      2: AFT.Tanh, 3: AFT.Sigmoid}
            vq = []
            c_prev = None
            for t in range(T):
                if t == 16:
                    ps_early.__exit__(None, None, None)
                    vpsum[0] = ps_voc.__enter__()
                ci = 0 if t < 8 else (1 if t < 24 else 2)
                tc0 = (t - XCH[ci][0]) * BC
                hdst, trel = hid_of(t)
                if t > 0:
                    hprev, tprel = hid_of(t - 1)
                psg = pG.tile([128, NGT, BC], F32, space="PSUM", tag="gates")
                gdst = {}
                for gi in GO:
                    for m in range(4):
                        g_i = gi * 4 + m
                        reg = psg[:, g_i, :]
                        nc.tensor.matmul(
                            reg, lhsT=identb[:],
                            rhs=xgc[ci][:, g_i, tc0:tc0 + BC],
                            start=True, stop=(t == 0))
                        if t > 0:
                            for k in range(4):
                                nc.tensor.matmul(
                                    reg, lhsT=WhhT_sb[:, k, g_i, :],
                                    rhs=hprev[:, k, tprel, :],
                                    start=False, stop=(k == 3))
                    dst = state.tile([128, 4, BC], F32, tag=f"act{gi}",
                                     name=f"act{gi}")
                    nc.scalar.activation(
                        dst[:], psg[:, gi * 4:(gi + 1) * 4, :], act_fn[gi])
                    gdst[gi] = dst

                c_cur = state.tile([128, 4, BC], F32, tag=f"c{t % 2}",
                                   name=f"c{t % 2}")
                if t == 0:
                    nc.vector.tensor_mul(c_cur[:], gdst[0][:], gdst[2][:])
                else:
                    ig = state.tile([128, 4, BC], F32, tag="ig")
                    nc.vector.tensor_mul(ig[:], gdst[0][:], gdst[2][:])
                    cf = state.tile([128, 4, BC], F32, tag="cf")
                    nc.gpsimd.tensor_mul(cf[:], gdst[1][:], c_prev[:])
                    nc.vector.tensor_add(c_cur[:], cf[:], ig[:])
                tanhc = state.tile([128, 4, BC], F32, tag="tanhc")
                nc.scalar.activation(tanhc[:], c_cur[:], AFT.Tanh)
                nc.vector.tensor_mul(
                    hdst[:, :, trel, :], gdst[3][:], tanhc[:])
                c_prev = c_cur

                # interleaved filler work
                if t < 8:
                    emit_xg(1, 2 * t)
                    emit_xg(1, 2 * t + 1)
                elif t < 16:
                    emit_xg(2, 2 * (t - 8))
                    emit_xg(2, 2 * (t - 8) + 1)
                for w, (t0, t1) in enumerate(VWIN):
                    if t == t1 - 1:
                        vq.extend((vt, w) for vt in range(NVT))
                if t >= 16 and vq:
                    emit_vunit(*vq.pop(0))

            # vocab tail
            while vq:
                emit_vunit(*vq.pop(0))

            ps_voc.__exit__(None, None, None)
            ps_gates.__exit__(None, None, None)

    nc.compile()
    _CACHE["nc"] = nc
    return nc


def kernel(features, seqs, lengths, W_in, b_in, emb, W_ih, W_hh, b_ih, b_hh,
           W_out, b_out):
    f32 = lambda x: np.asarray(x, dtype=np.float32)
    bf = lambda x: np.ascontiguousarray(f32(x)).astype(bfnp)
    features, seqs = f32(features), np.asarray(seqs).astype(np.int64)
    WinT = np.ascontiguousarray(
        bf(f32(W_in).T).reshape(12, 128, 4, 128).transpose(1, 0, 2, 3))
    WihT = np.ascontiguousarray(
        bf(f32(W_ih).T).reshape(4, 128, NGT, 128).transpose(1, 0, 2, 3))
    WhhT = np.ascontiguousarray(
        bf(f32(W_hh).T).reshape(4, 128, NGT, 128).transpose(1, 0, 2, 3))
    bcomb = np.ascontiguousarray(f32(b_ih) + f32(b_hh))
    emb_b = bf(emb)
    WoutT = np.zeros((H, VP), dtype=bfnp)
    WoutT[:, :V] = bf(f32(W_out).T)
    WoutTt = np.ascontiguousarray(
        WoutT.reshape(4, 128, NVT, 128).transpose(1, 0, 2, 3))
    ident_np = np.eye(128, dtype=bfnp)
    binp = f32(b_in)

    nc = _build()
    in_maps = []
    for c in range(NCORES):
        bs = slice(c * BC, (c + 1) * BC)
        featT = bf(features[bs].T)             # [F, BC]
        idx = np.zeros((NB,), np.int64)
        idx[BC:] = seqs[bs].T.reshape(-1)      # col c = t*BC + b, t>=1
        in_maps.append({
            "featT": featT,
            "idx": idx.reshape(NB, 1).astype(np.int32),
            "embt": emb_b,
            "WinT": WinT, "WihT": WihT, "WhhT": WhhT, "WoutT": WoutTt,
            "bcomb": bcomb, "bin": binp, "ident": ident_np,
        })
    _CACHE["last_in_maps"] = in_maps
    res = run_bass_kernel_spmd(nc, in_maps, list(range(NCORES)))
    out = np.empty((B, T, V), np.float32)
    for c in range(NCORES):
        parts = [
            np.asarray(res.results[c][f"out{w}"]).reshape(VP, -1)[:V]
            for w in range(3)
        ]
        lt = np.concatenate(parts, axis=1).astype(np.float32)  # [V, 640]
        out[c * BC:(c + 1) * BC] = (
            lt.reshape(V, T, BC).transpose(2, 1, 0))
    bo = f32(b_out)
    if np.any(bo):
        out += bo
    return out
